# revision 3
# baseline (speedup 1.0000x reference)
"""Multi-head attention with additive positional attention — TRN2 Bass kernel, v2.

Problem: B=4, S=2048, DM=128, H=8, DK=16.
  scores = (q @ k^T) / sqrt(DK) + pos_q @ pos_k^T   per (b, h)
  out    = softmax(scores) @ v, heads merged, @ Wo^T + bo

Sharding: 8 cores = batch (4) x query-row halves (2). Each core holds one
batch's full keys/values (S=2048) and R=1024 query rows, computes all 8
heads, and produces complete output rows; host gather is a concatenation.

v2 design vs v1:
  - All matmuls run as float32r (bitcast view): 1 cycle/row at N>=256 on the
    PE vs 4 for plain fp32, numerically fp32.
  - kcat/qcat (the per-head [k_h;pos_k_h] 32-row blocks) are produced
    DIRECTLY by two accumulating matmuls with host-permuted weight copies —
    no SBUF->SBUF partition-interleave DMAs.
  - v_aug ([1|v_h|0*15] per head, so attn@v also emits softmax row-sums and
    hard zeros) gets its bias and the ones-column from a second K=1 matmul
    accumulating into the same PSUM bank — no elementwise bias pass.
  - attn@v accumulates across the 16 key chunks IN PSUM (per-element
    has_written accumulate onto a memset-zeroed bank) — the per-chunk DVE
    adds of v1 are gone.
  - softmax denominators are partition-broadcast with a 0/1 selector matmul
    (bsel) instead of a DRAM DMA round-trip.
  - exp() is split: ScalarE (exact table exp) takes most columns; the DVE
    (+one GpSimd bit op) computes the rest with a Schraudolph bit-trick +
    quadratic mantissa correction (max rel err 0.64%), balancing the two
    engines. Set CD=0 to disable.
"""

import numpy as np

H, DK, DM = 8, 16, 128
B, S = 4, 2048
R = 1024  # query rows per core
NCORES = 8
NKC = S // 128  # 16 key chunks
QCW = 512  # query chunk width
NQC = R // QCW  # 2

# exp-approx constants
_L2E = 1.4426950408889634
A_CONST = float(2**23) * _L2E
B_CONST = float(127 * 2**23)
_w = np.linspace(1, 2, 200001)[:-1]
_g = 2.0 ** (_w - 1) / _w
C2, C1, C0 = [float(c) for c in np.polyfit(_w, _g, 2)]

CD = 288  # columns of each B score tile handled by the DVE exp chain
CD_WARM = 8  # iterations at loop head that run ACT-only (prologue drains DVE)

_CACHE = {}


def _patch_drain():
    """walrus on this stack rejects >1 sync-wait on CTRL instructions; the
    TileContext exit drain can carry several. Absorb them on SP nops first."""
    import concourse.mybir as mybir
    from concourse.tile import TileContext, ScopedClock

    if getattr(TileContext, "_drain_patched", False):
        return
    orig = TileContext._drain_and_barrier

    def patched(self, tick_clock, wait_clock):
        nc = self.nc
        probe = nc.sync.nop(nofuse=True)
        wait_clock.add_sem_waits(
            probe.ins, ScopedClock({None: tick_clock.global_clock})
        )
        w = list(probe.ins.sync_info.on_wait or []) if probe.ins.sync_info else []
        if len(w) > 1:
            probe.ins.sync_info.on_wait = w[:1]
            for i in range(1, len(w)):
                n2 = nc.sync.nop(nofuse=True)
                n2.ins.sync_info = mybir.SyncInfo(on_wait=w[i : i + 1], on_update=[])

        class _NoWaits:
            def __init__(s, real):
                s._real = real

            def add_sem_waits(s, ins, clock):
                pass

            def __getattr__(s, k):
                return getattr(s._real, k)

        orig(self, tick_clock, _NoWaits(wait_clock))

    TileContext._drain_and_barrier = patched
    TileContext._drain_patched = True


def _split_multi_waits(nc, mybir):
    """walrus here accepts at most 1 sync-wait on most instruction structs
    (2 on EventSemaphore). Hoist excess waits onto same-engine NoOps placed
    immediately before the instruction — same blocking semantics."""
    for f in nc.m.functions:
        for blk in f.blocks:
            new_insts = []
            changed = False
            for inst in blk.instructions:
                si = inst.sync_info
                waits = list(si.on_wait) if si and si.on_wait else []
                limit = 2 if type(inst).__name__ == "InstEventSemaphore" else 1
                if len(waits) > limit:
                    changed = True
                    extra = waits[: len(waits) - limit]
                    for wv in extra:
                        n = mybir.InstNoOp(
                            name=f"wsplit_{nc.next_id()}",
                            engine=inst.engine,
                            ins=[],
                            outs=[],
                            sync_info=mybir.SyncInfo(on_wait=[wv], on_update=[]),
                        )
                        nc.register_instruction(n)
                        new_insts.append(n)
                    inst.sync_info.on_wait = waits[len(waits) - limit :]
                new_insts.append(inst)
            if changed:
                blk.instructions = new_insts


def build_bass(mm_dtype="float32r", cd=CD):
    import concourse.bass as bass
    import concourse.mybir as mybir
    import concourse.tile as tile

    _patch_drain()
    dt = mybir.dt
    f32 = dt.float32
    f32r = dt.float32r
    bf16 = dt.bfloat16
    i32 = dt.int32
    mmdt = getattr(dt, mm_dtype)
    AF = mybir.ActivationFunctionType
    OP = mybir.AluOpType

    def mc(ap):  # matmul operands are float32r-typed end-to-end
        return ap

    nc = bass.Bass("TRN2", num_devices=NCORES, enable_asserts=True)

    def inp(name, shape):
        return nc.dram_tensor(name, shape, f32r, kind="ExternalInput")

    # packed inputs: few big DMAs instead of ~36 serialized small ones
    wpack_d = inp("wpack", [DM, 1669])
    rowpack_d = inp("rowpack", [1, 384])
    qpack_d = inp("qpack", [DM, 2 * R])
    kpack_d = inp("kpack", [DM, 3 * S])
    outT_d = nc.dram_tensor("outT", [DM, R], f32, kind="ExternalOutput")

    with tile.TileContext(nc) as tc:
        with (
            tc.tile_pool(name="singles", bufs=1) as singles,
            tc.tile_pool(name="escr", bufs=3) as escr,
            tc.tile_pool(name="chain", bufs=2) as chain,
            tc.tile_pool(name="nrm", bufs=2) as nrm,
            tc.tile_pool(name="ps", bufs=2, space="PSUM") as ps,
            tc.tile_pool(name="aux", bufs=2, space="PSUM") as aux,
            tc.tile_pool(name="accps", bufs=2, space="PSUM") as accps,
        ):
            # ---------------- input loads ----------------
            def dtile(name, shape, dram, col0):
                t = singles.tile(shape, f32r, tag=name, name=name)
                nc.sync.dma_start(
                    out=t[:, :], in_=dram[:, col0 : col0 + shape[-1]]
                )
                return t

            s_wpack = dtile("wpack_s", [DM, 1669], wpack_d, 0)
            sw = {}
            for g in (0, 1):
                for i, nm in enumerate(("w1A", "w1B", "w0A", "w0B", "woP")):
                    c0 = (5 * g + i) * DM
                    sw[f"{nm}{g}"] = s_wpack[:, c0 : c0 + DM]
            s_w2P = s_wpack[:, 1280:1536]
            s_bsel = s_wpack[:, 1536:1664]
            sw["b1c0"] = s_wpack[:, 1664:1665].bitcast(f32)
            sw["b0c0"] = s_wpack[:, 1665:1666].bitcast(f32)
            sw["b1c1"] = s_wpack[:, 1666:1667].bitcast(f32)
            sw["b0c1"] = s_wpack[:, 1667:1668].bitcast(f32)
            s_boc = s_wpack[:, 1668:1669].bitcast(f32)
            s_rowpack = dtile("rowpack_s", [1, 384], rowpack_d, 0)
            s_ones = s_rowpack[:, 0:DM]
            s_b2r = s_rowpack[:, DM : DM + 2 * DM]
            s_xqT = dtile("xqT_s", [DM, R], qpack_d, 0)
            s_posqT = dtile("posqT_s", [DM, R], qpack_d, R)
            kp = [dtile(f"kp{c}", [DM, 1024], kpack_d, 1024 * c) for c in range(6)]

            def kslice(base, c0, w):  # base: 0 xkT, 1 posT, 2 xvT (S cols each)
                a = 2 * base + c0 // 1024
                o = c0 % 1024
                return kp[a][:, o : o + w]

            # ACT exp table warmup
            warm = singles.tile([DM, 1], f32, tag="warm", name="warm")
            nc.scalar.activation(out=warm[:, :], in_=s_boc, func=AF.Exp)

            kcat = [
                singles.tile([DM, S], f32r, tag=f"kcat{g}", name=f"kcat{g}")
                for g in (0, 1)
            ]
            qcat = [
                singles.tile([DM, R], f32r, tag=f"qcat{g}", name=f"qcat{g}")
                for g in (0, 1)
            ]
            v_aug = singles.tile([DM, NKC, 2 * DM], bf16, tag="vaug", name="vaug")
            xs = [
                singles.tile([DM, R], f32r, tag=f"xs{g}", name=f"xs{g}") for g in (0, 1)
            ]
            outT_sb = singles.tile([DM, R], f32, tag="outsb", name="outsb")

            # ---------------- projection emitters ----------------
            def emit_kcat_chunk(g, c):  # c: 512-col chunk of S
                pk = aux.tile([128, 512], f32, tag="aux", name=f"kc{g}_{c}")
                sl = slice(c * 512, (c + 1) * 512)
                nc.tensor.matmul(
                    out=pk[:, 0:512], lhsT=mc(sw[f"w1A{g}"]),
                    rhs=mc(kslice(0, c * 512, 512)), start=True, stop=False,
                )
                nc.tensor.matmul(
                    out=pk[:, 0:512], lhsT=mc(sw[f"w1B{g}"]),
                    rhs=mc(kslice(1, c * 512, 512)), start=False, stop=True,
                )
                nc.vector.tensor_scalar_add(
                    out=kcat[g][:, sl], in0=pk[:, 0:512], scalar1=sw[f"b1c{g}"]
                )

            def emit_qcat_chunk(g, c):  # c: 512-col chunk of R
                pq = aux.tile([128, 512], f32, tag="aux", name=f"qc{g}_{c}")
                sl = slice(c * 512, (c + 1) * 512)
                nc.tensor.matmul(
                    out=pq[:, 0:512], lhsT=mc(sw[f"w0A{g}"]),
                    rhs=mc(s_xqT[:, sl]), start=True, stop=False,
                )
                nc.tensor.matmul(
                    out=pq[:, 0:512], lhsT=mc(sw[f"w0B{g}"]),
                    rhs=mc(s_posqT[:, sl]), start=False, stop=True,
                )
                nc.vector.tensor_scalar_add(
                    out=qcat[g][:, sl], in0=pq[:, 0:512], scalar1=sw[f"b0c{g}"]
                )

            def emit_v_chunk(t):
                pv = aux.tile([128, 512], f32, tag="aux", name=f"v{t}")
                nc.tensor.matmul(
                    out=pv[:, 0 : 2 * DM],
                    lhsT=mc(kslice(2, t * 128, 128)),
                    rhs=mc(s_w2P), start=True, stop=False,
                )
                # ones^T(1x128) @ b2r(1x256): adds the per-column bias row and
                # the 1.0 denominator column into every seq row
                nc.tensor.matmul(
                    out=pv[:, 0 : 2 * DM], lhsT=mc(s_ones),
                    rhs=mc(s_b2r), start=False, stop=True,
                )
                nc.vector.tensor_copy(out=v_aug[:, t, :], in_=pv[:, 0 : 2 * DM])

            emitted = set()

            def emit_extra(step):
                """Stagger remaining projection work into the first kc loop."""
                for item in PRELOAD.get(step, ()):
                    if item in emitted:
                        continue
                    emitted.add(item)
                    kind, a, b_ = item
                    if kind == "v":
                        emit_v_chunk(a)
                    elif kind == "k":
                        emit_kcat_chunk(a, b_)
                    elif kind == "q":
                        emit_qcat_chunk(a, b_)

            PRELOAD = {}
            for t in range(4, NKC):
                PRELOAD.setdefault(t - 4, []).append(("v", t, None))
            for c in range(4):
                PRELOAD.setdefault(4 + c, []).append(("k", 1, c))
            for c in range(2):
                PRELOAD.setdefault(9 + c, []).append(("q", 1, c))

            # prologue: what the first iterations need
            for c in range(4):
                emit_kcat_chunk(0, c)
            for c in range(2):
                emit_qcat_chunk(0, c)
            for t in range(4):
                emit_v_chunk(t)

            # ---------------- attention ----------------
            it = 0
            for g in (0, 1):
                for qc in range(NQC):
                    acc = accps.tile([128, QCW], f32, tag="acc", name=f"acc{g}{qc}")
                    # Zero the values so the first flags=0 matmul write per
                    # row group is correct whether stale has_written bits make
                    # it accumulate (onto 0) or overwrite. Only the very first
                    # matmul opens the bank's accumulation group.
                    nc.vector.memset(acc[:, :], 0.0)
                    for kc in range(NKC):
                        tA = ps.tile([128, 1024], f32, tag="sc", name=f"sA{it}")
                        tB = ps.tile([128, 1024], f32, tag="sc", name=f"sB{it}")
                        for j in range(4):
                            tgt = tA if j < 2 else tB
                            nc.tensor.matmul(
                                out=tgt[:, 512 * (j % 2) : 512 * (j % 2) + 512],
                                lhsT=mc(
                                    kcat[g][32 * j : 32 * j + 32, kc * 128 : (kc + 1) * 128]
                                ),
                                rhs=mc(
                                    qcat[g][32 * j : 32 * j + 32, qc * QCW : (qc + 1) * QCW]
                                ),
                                start=True, stop=True,
                                tile_position=(32 * j, 0),
                            )
                        eA = escr.tile([128, 1024], bf16, tag="eA", name=f"eA{it}")
                        eB = escr.tile([128, 1024], bf16, tag="eB", name=f"eB{it}")
                        cdi = cd if it >= CD_WARM else 0
                        nc.scalar.activation(out=eA[:, :], in_=tA[:, :], func=AF.Exp)
                        nc.scalar.activation(
                            out=eB[:, : 1024 - cdi], in_=tB[:, : 1024 - cdi], func=AF.Exp
                        )
                        if cdi:
                            csl_ps = tB[:, 1024 - cdi : 1024]
                            csl_e = eB[:, 1024 - cdi : 1024]
                            yi = chain.tile([128, cd], i32, tag="yi", name=f"yi{it}")
                            wt = chain.tile([128, cd], i32, tag="wt", name=f"wt{it}")
                            tt = chain.tile([128, cd], f32, tag="tt", name=f"tt{it}")
                            st = chain.tile([128, cd], f32, tag="st", name=f"st{it}")
                            nc.vector.tensor_scalar(
                                out=yi[:, :cdi], in0=csl_ps, scalar1=A_CONST,
                                scalar2=B_CONST, op0=OP.mult, op1=OP.add,
                            )
                            nc.vector.tensor_scalar(
                                out=wt[:, :cdi], in0=yi[:, :cdi],
                                scalar1=0x007FFFFF, scalar2=0x3F800000,
                                op0=OP.bitwise_and, op1=OP.bitwise_or,
                            )
                            nc.vector.tensor_scalar(
                                out=tt[:, :cdi], in0=wt[:, :cdi].bitcast(f32),
                                scalar1=C2, scalar2=C1, op0=OP.mult, op1=OP.add,
                            )
                            nc.vector.tensor_tensor(
                                out=st[:, :cdi], in0=tt[:, :cdi],
                                in1=wt[:, :cdi].bitcast(f32), op=OP.mult,
                            )
                            nc.vector.scalar_tensor_tensor(
                                out=csl_e, in0=st[:, :cdi], scalar=C0,
                                in1=yi[:, :cdi].bitcast(f32), op0=OP.add, op1=OP.mult,
                            )
                        for j in range(4):
                            esrc = eA if j < 2 else eB
                            h = 4 * g + j
                            nc.tensor.matmul(
                                out=acc[32 * j : 32 * j + 32, :],
                                lhsT=mc(v_aug[:, kc, 32 * h : 32 * h + 32]),
                                rhs=mc(esrc[:, 512 * (j % 2) : 512 * (j % 2) + 512]),
                                start=(kc == 0 and j == 0),
                                stop=(kc == NKC - 1 and j == 3),
                                tile_position=(0, 32 * j),
                                skip_group_check=True,
                            )
                        if g == 0 and qc == 0:
                            emit_extra(kc)
                        it += 1

                    # -------- per-(g,qc) normalize into xs --------
                    acc_sb = nrm.tile([128, QCW], f32r, tag="accsb", name=f"as{g}{qc}")
                    nc.vector.tensor_copy(out=acc_sb[:, :], in_=acc[:, :])
                    Dp = aux.tile([128, 512], f32, tag="aux", name=f"D{g}{qc}")
                    nc.tensor.matmul(
                        out=Dp[:, 0:QCW], lhsT=mc(s_bsel), rhs=mc(acc_sb[:, :]),
                        start=True, stop=True,
                    )
                    rcp = nrm.tile([128, QCW], f32, tag="rcp", name=f"rc{g}{qc}")
                    nc.vector.reciprocal(out=rcp[:, :], in_=Dp[:, 0:QCW])
                    nc.vector.tensor_tensor(
                        out=xs[g][:, qc * QCW : (qc + 1) * QCW],
                        in0=acc_sb[:, :], in1=rcp[:, :], op=OP.mult,
                    )

            # ---------------- output projection ----------------
            for qc in range(NQC):
                sl = slice(qc * QCW, (qc + 1) * QCW)
                po = aux.tile([128, 512], f32, tag="aux", name=f"po{qc}")
                nc.tensor.matmul(
                    out=po[:, 0:QCW], lhsT=mc(sw["woP0"]), rhs=mc(xs[0][:, sl]),
                    start=True, stop=False,
                )
                nc.tensor.matmul(
                    out=po[:, 0:QCW], lhsT=mc(sw["woP1"]), rhs=mc(xs[1][:, sl]),
                    start=False, stop=True,
                )
                nc.vector.tensor_scalar_add(
                    out=outT_sb[:, sl], in0=po[:, 0:QCW], scalar1=s_boc
                )
            nc.sync.dma_start(out=outT_d[:, :], in_=outT_sb[:, :])

    import concourse.mybir as mybir

    _split_multi_waits(nc, mybir)
    return nc


def shard_inputs(query, key, value, pos_embed, W0, b0, W1, b1, W2, b2, Wo, bo):
    """Build the 8 per-core input maps (host-side layout preprocessing)."""
    f = np.float32
    asc = np.ascontiguousarray
    scale = 1.0 / np.sqrt(np.float32(DK))

    W0T = asc(np.asarray(W0).T.astype(f))
    W1T = asc(np.asarray(W1).T.astype(f))
    W2T = asc(np.asarray(W2).T.astype(f))
    WoT = asc(np.asarray(Wo).T.astype(f))
    b0, b1, b2, bo = [np.asarray(x).astype(f) for x in (b0, b1, b2, bo)]

    wpack = np.zeros((DM, 1669), f)
    for g in (0, 1):
        w1A = np.zeros((DM, DM), f)
        w1B = np.zeros((DM, DM), f)
        w0A = np.zeros((DM, DM), f)
        w0B = np.zeros((DM, DM), f)
        b1c = np.zeros((DM,), f)
        b0c = np.zeros((DM,), f)
        woP = np.zeros((DM, DM), f)
        for j in range(4):
            h = 4 * g + j
            hs = slice(16 * h, 16 * h + 16)
            w1A[:, 32 * j : 32 * j + 16] = W1T[:, hs]
            w1B[:, 32 * j + 16 : 32 * j + 32] = W1T[:, hs]
            w0A[:, 32 * j : 32 * j + 16] = W0T[:, hs] * scale
            w0B[:, 32 * j + 16 : 32 * j + 32] = W0T[:, hs]
            b1c[32 * j : 32 * j + 16] = b1[hs]
            b1c[32 * j + 16 : 32 * j + 32] = b1[hs]
            b0c[32 * j : 32 * j + 16] = b0[hs] * scale
            b0c[32 * j + 16 : 32 * j + 32] = b0[hs]
            woP[32 * j + 1 : 32 * j + 17, :] = WoT[hs, :]
        for i, w in enumerate((w1A, w1B, w0A, w0B, woP)):
            c0 = (5 * g + i) * DM
            wpack[:, c0 : c0 + DM] = w
        wpack[:, 1664 + 2 * g] = b1c
        wpack[:, 1665 + 2 * g] = b0c
    w2P = np.zeros((DM, 2 * DM), f)
    b2r = np.zeros((2 * DM,), f)
    bsel = np.zeros((DM, DM), f)
    for h in range(8):
        w2P[:, 32 * h + 1 : 32 * h + 17] = W2T[:, 16 * h : 16 * h + 16]
        b2r[32 * h] = 1.0
        b2r[32 * h + 1 : 32 * h + 17] = b2[16 * h : 16 * h + 16]
    for j in range(4):
        bsel[32 * j, 32 * j : 32 * j + 32] = 1.0
    wpack[:, 1280:1536] = w2P
    wpack[:, 1536:1664] = bsel
    wpack[:, 1668] = bo
    rowpack = np.zeros((1, 384), f)
    rowpack[0, 0:DM] = 1.0
    rowpack[0, DM : DM + 2 * DM] = b2r
    shared = {"wpack": wpack, "rowpack": rowpack}

    query, key, value, pos_embed = [
        np.asarray(x) for x in (query, key, value, pos_embed)
    ]
    in_maps = []
    for c in range(NCORES):
        b_i, half = divmod(c, 2)
        r0 = half * R
        qpk = np.empty((DM, 2 * R), f)
        qpk[:, 0:R] = query[b_i, r0 : r0 + R, :].T
        qpk[:, R : 2 * R] = pos_embed[b_i, r0 : r0 + R, :].T
        kpk = np.empty((DM, 3 * S), f)
        kpk[:, 0:S] = key[b_i].T
        kpk[:, S : 2 * S] = pos_embed[b_i].T
        kpk[:, 2 * S : 3 * S] = value[b_i].T
        in_maps.append(dict(shared, qpack=qpk, kpack=kpk))
    return in_maps


def gather_outputs(results):
    out = np.empty((B, S, DM), np.float32)
    for c in range(NCORES):
        b_i, half = divmod(c, 2)
        r0 = half * R
        out[b_i, r0 : r0 + R, :] = results[c]["outT"].T
    return out


def kernel(query, key, value, pos_embed, W0, b0, W1, b1, W2, b2, Wo, bo):
    from concourse.bass_utils import run_bass_kernel_spmd

    if "nc" not in _CACHE:
        _CACHE["nc"] = build_bass()
    in_maps = shard_inputs(
        query, key, value, pos_embed, W0, b0, W1, b1, W2, b2, Wo, bo
    )
    res = run_bass_kernel_spmd(_CACHE["nc"], in_maps, core_ids=list(range(NCORES)))
    return gather_outputs(res.results)


# revision 4
# speedup vs baseline: 1.0497x; 1.0497x over previous
"""Multi-head attention with additive positional attention — TRN2 Bass kernel, v2.

Problem: B=4, S=2048, DM=128, H=8, DK=16.
  scores = (q @ k^T) / sqrt(DK) + pos_q @ pos_k^T   per (b, h)
  out    = softmax(scores) @ v, heads merged, @ Wo^T + bo

Sharding: 8 cores = batch (4) x query-row halves (2). Each core holds one
batch's full keys/values (S=2048) and R=1024 query rows, computes all 8
heads, and produces complete output rows; host gather is a concatenation.

v2 design vs v1:
  - All matmuls run as float32r (bitcast view): 1 cycle/row at N>=256 on the
    PE vs 4 for plain fp32, numerically fp32.
  - kcat/qcat (the per-head [k_h;pos_k_h] 32-row blocks) are produced
    DIRECTLY by two accumulating matmuls with host-permuted weight copies —
    no SBUF->SBUF partition-interleave DMAs.
  - v_aug ([1|v_h|0*15] per head, so attn@v also emits softmax row-sums and
    hard zeros) gets its bias and the ones-column from a second K=1 matmul
    accumulating into the same PSUM bank — no elementwise bias pass.
  - attn@v accumulates across the 16 key chunks IN PSUM (per-element
    has_written accumulate onto a memset-zeroed bank) — the per-chunk DVE
    adds of v1 are gone.
  - softmax denominators are partition-broadcast with a 0/1 selector matmul
    (bsel) instead of a DRAM DMA round-trip.
  - exp() is split: ScalarE (exact table exp) takes most columns; the DVE
    (+one GpSimd bit op) computes the rest with a Schraudolph bit-trick +
    quadratic mantissa correction (max rel err 0.64%), balancing the two
    engines. Set CD=0 to disable.
"""

import numpy as np

H, DK, DM = 8, 16, 128
B, S = 4, 2048
R = 1024  # query rows per core
NCORES = 8
NKC = S // 128  # 16 key chunks
QCW = 512  # query chunk width
NQC = R // QCW  # 2

# exp-approx constants
_L2E = 1.4426950408889634
A_CONST = float(2**23) * _L2E
B_CONST = float(127 * 2**23)
_w = np.linspace(1, 2, 200001)[:-1]
_g = 2.0 ** (_w - 1) / _w
C2, C1, C0 = [float(c) for c in np.polyfit(_w, _g, 2)]

CD = 288  # columns of each B score tile handled by the DVE exp chain
CD_WARM = 8  # iterations at loop head that run ACT-only (prologue drains DVE)

_CACHE = {}


def _patch_drain():
    """walrus on this stack rejects >1 sync-wait on CTRL instructions; the
    TileContext exit drain can carry several. Absorb them on SP nops first."""
    import concourse.mybir as mybir
    from concourse.tile import TileContext, ScopedClock

    if getattr(TileContext, "_drain_patched", False):
        return
    orig = TileContext._drain_and_barrier

    def patched(self, tick_clock, wait_clock):
        nc = self.nc
        probe = nc.sync.nop(nofuse=True)
        wait_clock.add_sem_waits(
            probe.ins, ScopedClock({None: tick_clock.global_clock})
        )
        w = list(probe.ins.sync_info.on_wait or []) if probe.ins.sync_info else []
        if len(w) > 1:
            probe.ins.sync_info.on_wait = w[:1]
            for i in range(1, len(w)):
                n2 = nc.sync.nop(nofuse=True)
                n2.ins.sync_info = mybir.SyncInfo(on_wait=w[i : i + 1], on_update=[])

        class _NoWaits:
            def __init__(s, real):
                s._real = real

            def add_sem_waits(s, ins, clock):
                pass

            def __getattr__(s, k):
                return getattr(s._real, k)

        orig(self, tick_clock, _NoWaits(wait_clock))

    TileContext._drain_and_barrier = patched
    TileContext._drain_patched = True


def _split_multi_waits(nc, mybir):
    """walrus here accepts at most 1 sync-wait on most instruction structs
    (2 on EventSemaphore). Hoist excess waits onto same-engine NoOps placed
    immediately before the instruction — same blocking semantics."""
    for f in nc.m.functions:
        for blk in f.blocks:
            new_insts = []
            changed = False
            for inst in blk.instructions:
                si = inst.sync_info
                waits = list(si.on_wait) if si and si.on_wait else []
                limit = 2 if type(inst).__name__ == "InstEventSemaphore" else 1
                if len(waits) > limit:
                    changed = True
                    extra = waits[: len(waits) - limit]
                    for wv in extra:
                        n = mybir.InstNoOp(
                            name=f"wsplit_{nc.next_id()}",
                            engine=inst.engine,
                            ins=[],
                            outs=[],
                            sync_info=mybir.SyncInfo(on_wait=[wv], on_update=[]),
                        )
                        nc.register_instruction(n)
                        new_insts.append(n)
                    inst.sync_info.on_wait = waits[len(waits) - limit :]
                new_insts.append(inst)
            if changed:
                blk.instructions = new_insts


def build_bass(mm_dtype="float32r", cd=CD):
    import concourse.bass as bass
    import concourse.mybir as mybir
    import concourse.tile as tile

    _patch_drain()
    dt = mybir.dt
    f32 = dt.float32
    f32r = dt.float32r
    bf16 = dt.bfloat16
    i32 = dt.int32
    mmdt = getattr(dt, mm_dtype)
    AF = mybir.ActivationFunctionType
    OP = mybir.AluOpType

    def mc(ap):  # matmul operands are float32r-typed end-to-end
        return ap

    nc = bass.Bass("TRN2", num_devices=NCORES, enable_asserts=True)

    def inp(name, shape):
        return nc.dram_tensor(name, shape, f32r, kind="ExternalInput")

    # packed inputs: few big DMAs instead of ~36 serialized small ones
    wpack_d = inp("wpack", [DM, 1669])
    rowpack_d = inp("rowpack", [1, 384])
    qpack_d = inp("qpack", [DM, 2 * R])
    kpack_d = inp("kpack", [DM, 3 * S])
    outT_d = nc.dram_tensor("outT", [DM, R], f32, kind="ExternalOutput")

    with tile.TileContext(nc) as tc:
        with (
            tc.tile_pool(name="singles", bufs=1) as singles,
            tc.tile_pool(name="escr", bufs=5) as escr,
            tc.tile_pool(name="chain", bufs=3) as chain,
            tc.tile_pool(name="nrm", bufs=2) as nrm,
            tc.tile_pool(name="ps", bufs=3, space="PSUM") as ps,
            tc.tile_pool(name="aux", bufs=1, space="PSUM") as aux,
            tc.tile_pool(name="accps", bufs=1, space="PSUM") as accps,
        ):
            # ---------------- input loads ----------------
            def dtile(name, shape, dram, col0, eng=None):
                t = singles.tile(shape, f32r, tag=name, name=name)
                (eng or nc.sync).dma_start(
                    out=t[:, :], in_=dram[:, col0 : col0 + shape[-1]]
                )
                return t

            s_wpack = dtile("wpack_s", [DM, 1669], wpack_d, 0)
            sw = {}
            for g in (0, 1):
                for i, nm in enumerate(("w1A", "w1B", "w0A", "w0B", "woP")):
                    c0 = (5 * g + i) * DM
                    sw[f"{nm}{g}"] = s_wpack[:, c0 : c0 + DM]
            s_w2P = s_wpack[:, 1280:1536]
            s_bsel = s_wpack[:, 1536:1664]
            sw["b1c0"] = s_wpack[:, 1664:1665].bitcast(f32)
            sw["b0c0"] = s_wpack[:, 1665:1666].bitcast(f32)
            sw["b1c1"] = s_wpack[:, 1666:1667].bitcast(f32)
            sw["b0c1"] = s_wpack[:, 1667:1668].bitcast(f32)
            s_boc = s_wpack[:, 1668:1669].bitcast(f32)
            # split loads across the two HWDGE queues (SP + Activation) so
            # transfers run in parallel; order by first use in the pipeline
            s_rowpack = dtile("rowpack_s", [1, 384], rowpack_d, 0, eng=nc.scalar)
            s_ones = s_rowpack[:, 0:DM]
            s_b2r = s_rowpack[:, DM : DM + 2 * DM]
            kp = [None] * 6
    
            kp[0] = dtile("kp0", [DM, 1024], kpack_d, 0)  # SP: xk 0:1024
            kp[2] = dtile("kp2", [DM, 1024], kpack_d, 2048, eng=nc.scalar)
            s_xqT = dtile("xqT_s", [DM, R], qpack_d, 0, eng=nc.scalar)
            s_posqT = dtile("posqT_s", [DM, R], qpack_d, R, eng=nc.scalar)
            kp[4] = dtile("kp4", [DM, 1024], kpack_d, 4096)  # SP: xv 0:1024
            kp[1] = dtile("kp1", [DM, 1024], kpack_d, 1024)
            kp[3] = dtile("kp3", [DM, 1024], kpack_d, 3072, eng=nc.scalar)
            kp[5] = dtile("kp5", [DM, 1024], kpack_d, 5120)

            def kslice(base, c0, w):  # base: 0 xkT, 1 posT, 2 xvT (S cols each)
                a = 2 * base + c0 // 1024
                o = c0 % 1024
                return kp[a][:, o : o + w]

            # ACT exp table warmup
            warm = singles.tile([DM, 1], f32, tag="warm", name="warm")
            nc.scalar.activation(out=warm[:, :], in_=s_boc, func=AF.Exp)

            kcat = [
                singles.tile([DM, S], f32r, tag=f"kcat{g}", name=f"kcat{g}")
                for g in (0, 1)
            ]
            qcat = [
                singles.tile([DM, R], f32r, tag=f"qcat{g}", name=f"qcat{g}")
                for g in (0, 1)
            ]
            v_aug = singles.tile([DM, NKC, 2 * DM], bf16, tag="vaug", name="vaug")
            xs = [
                singles.tile([DM, R], f32r, tag=f"xs{g}", name=f"xs{g}") for g in (0, 1)
            ]
            outT_sb = singles.tile([DM, R], f32, tag="outsb", name="outsb")

            # ---------------- projection emitters ----------------
            def emit_kcat_chunk(g, c):  # c: 512-col chunk of S
                pk = aux.tile([128, 512], f32, tag="aux", name=f"kc{g}_{c}")
                sl = slice(c * 512, (c + 1) * 512)
                nc.tensor.matmul(
                    out=pk[:, 0:512], lhsT=mc(sw[f"w1A{g}"]),
                    rhs=mc(kslice(0, c * 512, 512)), start=True, stop=False,
                )
                nc.tensor.matmul(
                    out=pk[:, 0:512], lhsT=mc(sw[f"w1B{g}"]),
                    rhs=mc(kslice(1, c * 512, 512)), start=False, stop=True,
                )
                nc.vector.tensor_scalar_add(
                    out=kcat[g][:, sl], in0=pk[:, 0:512], scalar1=sw[f"b1c{g}"]
                )

            def emit_qcat_chunk(g, c):  # c: 512-col chunk of R
                pq = aux.tile([128, 512], f32, tag="aux", name=f"qc{g}_{c}")
                sl = slice(c * 512, (c + 1) * 512)
                nc.tensor.matmul(
                    out=pq[:, 0:512], lhsT=mc(sw[f"w0A{g}"]),
                    rhs=mc(s_xqT[:, sl]), start=True, stop=False,
                )
                nc.tensor.matmul(
                    out=pq[:, 0:512], lhsT=mc(sw[f"w0B{g}"]),
                    rhs=mc(s_posqT[:, sl]), start=False, stop=True,
                )
                nc.vector.tensor_scalar_add(
                    out=qcat[g][:, sl], in0=pq[:, 0:512], scalar1=sw[f"b0c{g}"]
                )

            def emit_v_chunk(t):
                pv = aux.tile([128, 512], f32, tag="aux", name=f"v{t}")
                nc.tensor.matmul(
                    out=pv[:, 0 : 2 * DM],
                    lhsT=mc(kslice(2, t * 128, 128)),
                    rhs=mc(s_w2P), start=True, stop=False,
                )
                # ones^T(1x128) @ b2r(1x256): adds the per-column bias row and
                # the 1.0 denominator column into every seq row
                nc.tensor.matmul(
                    out=pv[:, 0 : 2 * DM], lhsT=mc(s_ones),
                    rhs=mc(s_b2r), start=False, stop=True,
                )
                nc.vector.tensor_copy(out=v_aug[:, t, :], in_=pv[:, 0 : 2 * DM])

            emitted = set()

            def emit_extra(step):
                """Stagger remaining projection work into the first kc loop."""
                for item in PRELOAD.get(step, ()):
                    if item in emitted:
                        continue
                    emitted.add(item)
                    kind, a, b_ = item
                    if kind == "v":
                        emit_v_chunk(a)
                    elif kind == "k":
                        emit_kcat_chunk(a, b_)
                    elif kind == "q":
                        emit_qcat_chunk(a, b_)

            PRELOAD = {}
            for t in range(4, NKC):
                PRELOAD.setdefault(t - 4, []).append(("v", t, None))
            for c in range(4):
                PRELOAD.setdefault(4 + c, []).append(("k", 1, c))
            for c in range(2):
                PRELOAD.setdefault(9 + c, []).append(("q", 1, c))

            # prologue: what the first iterations need
            for c in range(4):
                emit_kcat_chunk(0, c)
            for c in range(2):
                emit_qcat_chunk(0, c)
            for t in range(4):
                emit_v_chunk(t)

            # ---------------- attention ----------------
            it = 0
            for g in (0, 1):
                for qc in range(NQC):
                    acc = accps.tile([128, QCW], f32, tag="acc", name=f"acc{g}{qc}")
                    # Zero the values so the first flags=0 matmul write per
                    # row group is correct whether stale has_written bits make
                    # it accumulate (onto 0) or overwrite. Only the very first
                    # matmul opens the bank's accumulation group.
                    nc.vector.memset(acc[:, :], 0.0)
                    pend_av = []
                    pend_stt = [None]
                    for kc in range(NKC):
                        tA = ps.tile([128, 1024], f32, tag="sc", name=f"sA{it}")
                        tB = ps.tile([128, 1024], f32, tag="sc", name=f"sB{it}")
                        for j in range(4):
                            tgt = tA if j < 2 else tB
                            nc.tensor.matmul(
                                out=tgt[:, 512 * (j % 2) : 512 * (j % 2) + 512],
                                lhsT=mc(
                                    kcat[g][32 * j : 32 * j + 32, kc * 128 : (kc + 1) * 128]
                                ),
                                rhs=mc(
                                    qcat[g][32 * j : 32 * j + 32, qc * QCW : (qc + 1) * QCW]
                                ),
                                start=True, stop=True,
                                tile_position=(32 * j, 0),
                            )
                        if len(pend_av) >= 2:
                            pend_av.pop(0)()
                        eA = escr.tile([128, 1024], bf16, tag="eA", name=f"eA{it}")
                        eB = escr.tile([128, 1024], bf16, tag="eB", name=f"eB{it}")
                        cdi = cd if it >= CD_WARM else 0
                        nc.scalar.activation(out=eA[:, :], in_=tA[:, :], func=AF.Exp)
                        nc.scalar.activation(
                            out=eB[:, : 1024 - cdi], in_=tB[:, : 1024 - cdi], func=AF.Exp
                        )
                        if pend_stt[0] is not None:
                            # previous iteration's final approx-exp multiply:
                            # deferred one iteration so the DVE FIFO never
                            # stalls on the Pool-engine s-step round trip
                            pend_stt[0]()
                            pend_stt[0] = None
                        if cdi:
                            csl_ps = tB[:, 1024 - cdi : 1024]
                            csl_e = eB[:, 1024 - cdi : 1024]
                            yi = chain.tile([128, cd], i32, tag="yi", name=f"yi{it}")
                            wt = chain.tile([128, cd], i32, tag="wt", name=f"wt{it}")
                            tt = chain.tile([128, cd], f32, tag="tt", name=f"tt{it}")
                            st = chain.tile([128, cd], f32, tag="st", name=f"st{it}")
                            nc.vector.tensor_scalar(
                                out=yi[:, :cdi], in0=csl_ps, scalar1=A_CONST,
                                scalar2=B_CONST, op0=OP.mult, op1=OP.add,
                            )
                            nc.vector.tensor_scalar(
                                out=wt[:, :cdi], in0=yi[:, :cdi],
                                scalar1=0x007FFFFF, scalar2=0x3F800000,
                                op0=OP.bitwise_and, op1=OP.bitwise_or,
                            )
                            nc.vector.tensor_scalar(
                                out=tt[:, :cdi], in0=wt[:, :cdi].bitcast(f32),
                                scalar1=C2, scalar2=C1, op0=OP.mult, op1=OP.add,
                            )
                            nc.gpsimd.tensor_tensor(
                                out=st[:, :cdi], in0=tt[:, :cdi],
                                in1=wt[:, :cdi].bitcast(f32), op=OP.mult,
                            )

                            def make_stt(cdi, st, yi, csl_e):
                                def emit():
                                    nc.vector.scalar_tensor_tensor(
                                        out=csl_e, in0=st[:, :cdi], scalar=C0,
                                        in1=yi[:, :cdi].bitcast(f32),
                                        op0=OP.add, op1=OP.mult,
                                    )
                                return emit

                            pend_stt[0] = make_stt(cdi, st, yi, csl_e)
                        def make_av(kc, eA, eB):
                            def emit():
                                for j in range(4):
                                    esrc = eA if j < 2 else eB
                                    h = 4 * g + j
                                    nc.tensor.matmul(
                                        out=acc[32 * j : 32 * j + 32, :],
                                        lhsT=mc(v_aug[:, kc, 32 * h : 32 * h + 32]),
                                        rhs=mc(
                                            esrc[:, 512 * (j % 2) : 512 * (j % 2) + 512]
                                        ),
                                        start=(kc == 0 and j == 0),
                                        stop=(kc == NKC - 1 and j == 3),
                                        tile_position=(0, 32 * j),
                                        skip_group_check=True,
                                    )
                            return emit

                        pend_av.append(make_av(kc, eA, eB))
                        if g == 0 and qc == 0:
                            emit_extra(kc)
                        it += 1
                    if pend_stt[0] is not None:
                        pend_stt[0]()
                        pend_stt[0] = None
                    for f_ in pend_av:
                        f_()

                    # -------- per-(g,qc) normalize into xs --------
                    acc_sb = nrm.tile([128, QCW], f32r, tag="accsb", name=f"as{g}{qc}")
                    nc.vector.tensor_copy(out=acc_sb[:, :], in_=acc[:, :])
                    Dp = aux.tile([128, 512], f32, tag="aux", name=f"D{g}{qc}")
                    nc.tensor.matmul(
                        out=Dp[:, 0:QCW], lhsT=mc(s_bsel), rhs=mc(acc_sb[:, :]),
                        start=True, stop=True,
                    )
                    rcp = nrm.tile([128, QCW], f32, tag="rcp", name=f"rc{g}{qc}")
                    nc.vector.reciprocal(out=rcp[:, :], in_=Dp[:, 0:QCW])
                    nc.vector.tensor_tensor(
                        out=xs[g][:, qc * QCW : (qc + 1) * QCW],
                        in0=acc_sb[:, :], in1=rcp[:, :], op=OP.mult,
                    )
                    if g == 1:
                        # output projection for this query chunk overlaps the
                        # remaining attention blocks / drains
                        sl = slice(qc * QCW, (qc + 1) * QCW)
                        po = aux.tile([128, 512], f32, tag="aux", name=f"po{qc}")
                        nc.tensor.matmul(
                            out=po[:, 0:QCW], lhsT=mc(sw["woP0"]),
                            rhs=mc(xs[0][:, sl]), start=True, stop=False,
                        )
                        nc.tensor.matmul(
                            out=po[:, 0:QCW], lhsT=mc(sw["woP1"]),
                            rhs=mc(xs[1][:, sl]), start=False, stop=True,
                        )
                        nc.vector.tensor_scalar_add(
                            out=outT_sb[:, sl], in0=po[:, 0:QCW], scalar1=s_boc
                        )
                        nc.sync.dma_start(out=outT_d[:, sl], in_=outT_sb[:, sl])



    import concourse.mybir as mybir

    _split_multi_waits(nc, mybir)
    return nc


def shard_inputs(query, key, value, pos_embed, W0, b0, W1, b1, W2, b2, Wo, bo):
    """Build the 8 per-core input maps (host-side layout preprocessing)."""
    f = np.float32
    asc = np.ascontiguousarray
    scale = 1.0 / np.sqrt(np.float32(DK))

    W0T = asc(np.asarray(W0).T.astype(f))
    W1T = asc(np.asarray(W1).T.astype(f))
    W2T = asc(np.asarray(W2).T.astype(f))
    WoT = asc(np.asarray(Wo).T.astype(f))
    b0, b1, b2, bo = [np.asarray(x).astype(f) for x in (b0, b1, b2, bo)]

    wpack = np.zeros((DM, 1669), f)
    for g in (0, 1):
        w1A = np.zeros((DM, DM), f)
        w1B = np.zeros((DM, DM), f)
        w0A = np.zeros((DM, DM), f)
        w0B = np.zeros((DM, DM), f)
        b1c = np.zeros((DM,), f)
        b0c = np.zeros((DM,), f)
        woP = np.zeros((DM, DM), f)
        for j in range(4):
            h = 4 * g + j
            hs = slice(16 * h, 16 * h + 16)
            w1A[:, 32 * j : 32 * j + 16] = W1T[:, hs]
            w1B[:, 32 * j + 16 : 32 * j + 32] = W1T[:, hs]
            w0A[:, 32 * j : 32 * j + 16] = W0T[:, hs] * scale
            w0B[:, 32 * j + 16 : 32 * j + 32] = W0T[:, hs]
            b1c[32 * j : 32 * j + 16] = b1[hs]
            b1c[32 * j + 16 : 32 * j + 32] = b1[hs]
            b0c[32 * j : 32 * j + 16] = b0[hs] * scale
            b0c[32 * j + 16 : 32 * j + 32] = b0[hs]
            woP[32 * j + 1 : 32 * j + 17, :] = WoT[hs, :]
        for i, w in enumerate((w1A, w1B, w0A, w0B, woP)):
            c0 = (5 * g + i) * DM
            wpack[:, c0 : c0 + DM] = w
        wpack[:, 1664 + 2 * g] = b1c
        wpack[:, 1665 + 2 * g] = b0c
    w2P = np.zeros((DM, 2 * DM), f)
    b2r = np.zeros((2 * DM,), f)
    bsel = np.zeros((DM, DM), f)
    for h in range(8):
        w2P[:, 32 * h + 1 : 32 * h + 17] = W2T[:, 16 * h : 16 * h + 16]
        b2r[32 * h] = 1.0
        b2r[32 * h + 1 : 32 * h + 17] = b2[16 * h : 16 * h + 16]
    for j in range(4):
        bsel[32 * j, 32 * j : 32 * j + 32] = 1.0
    wpack[:, 1280:1536] = w2P
    wpack[:, 1536:1664] = bsel
    wpack[:, 1668] = bo
    rowpack = np.zeros((1, 384), f)
    rowpack[0, 0:DM] = 1.0
    rowpack[0, DM : DM + 2 * DM] = b2r
    shared = {"wpack": wpack, "rowpack": rowpack}

    query, key, value, pos_embed = [
        np.asarray(x) for x in (query, key, value, pos_embed)
    ]
    in_maps = []
    for c in range(NCORES):
        b_i, half = divmod(c, 2)
        r0 = half * R
        qpk = np.empty((DM, 2 * R), f)
        qpk[:, 0:R] = query[b_i, r0 : r0 + R, :].T
        qpk[:, R : 2 * R] = pos_embed[b_i, r0 : r0 + R, :].T
        kpk = np.empty((DM, 3 * S), f)
        kpk[:, 0:S] = key[b_i].T
        kpk[:, S : 2 * S] = pos_embed[b_i].T
        kpk[:, 2 * S : 3 * S] = value[b_i].T
        in_maps.append(dict(shared, qpack=qpk, kpack=kpk))
    return in_maps


def gather_outputs(results):
    out = np.empty((B, S, DM), np.float32)
    for c in range(NCORES):
        b_i, half = divmod(c, 2)
        r0 = half * R
        out[b_i, r0 : r0 + R, :] = results[c]["outT"].T
    return out


def kernel(query, key, value, pos_embed, W0, b0, W1, b1, W2, b2, Wo, bo):
    from concourse.bass_utils import run_bass_kernel_spmd

    if "nc" not in _CACHE:
        _CACHE["nc"] = build_bass()
    in_maps = shard_inputs(
        query, key, value, pos_embed, W0, b0, W1, b1, W2, b2, Wo, bo
    )
    res = run_bass_kernel_spmd(_CACHE["nc"], in_maps, core_ids=list(range(NCORES)))
    return gather_outputs(res.results)
